# revision 14
# baseline (speedup 1.0000x reference)
"""Trainium2 Bass kernel for nn_CLOSpreadModel (bucketed hinge ensemble).

out = hinge(mvoc; base) + bucket_adj(mvoc, bucket_idx) + hinge(lev_idx)
    + hinge(wap) + hinge(cpnspread) + hinge(equity_nav) + bias
with hinge(x) = b + sum_k w_k * relu(x - t_k).

Algorithm (v4, piecewise-linear + bucket-sorted sharding):
  Sharding: elements are stable-sorted by bucket_idx on the host (MoE-style
  dispatch done at the sharding/layout level; the inverse permutation is
  applied to the output). Each of the 1024 (core, partition) slots then
  holds 2048 consecutive sorted elements, spanning at most 2 buckets.

  Part A (mvoc base + bucket adjustment): both are piecewise-linear in mvoc,
  so each slot evaluates them with ONE value-table lookup per row:
      idx = slotbit*NB + round((mvoc - lo)/delta),   T_slot[idx]
  where T_slot = [table(bucket A) | table(bucket B)] for the slot's (at
  most) two buckets, tabulated on an NB-point uniform grid over the actual
  mvoc range, with every additive constant of the model folded in. The
  per-row gather runs on GPSIMD via indirect_copy (16-partition groups
  share an index stream; the host pre-permutes mvoc and the slotbit tile so
  the elementwise index tile IS the wrapped index layout, and the gather
  output de-replicates back to natural layout with strided-partition DMAs).

  Part B (lev/wap/cpn/nav): each 32-knot hinge is re-fit over the actual
  data range with a Ramer-Douglas-Peucker pass (L_inf <= EPS), cutting it
  to ~9-12 knots. Each knot is ONE 4x-rate DVE pass via
      max(|w|*x, |w|*t) = |w|*relu(x - t) + |w|*t
  (constants folded into the table). The Tensor engine accumulates the
  bf16 term tiles into PSUM with +/-identity matmuls (sign of w).

  Final: y = psum + gathered  (one DVE add), DMA out f32, host unsorts.
"""
import numpy as np
from contextlib import ExitStack

import concourse.bass as bass
import concourse.mybir as mybir
from concourse.bass_utils import run_bass_kernel_spmd

ALU = mybir.AluOpType
DT = mybir.dt

N = 2_097_152
NCORES = 8
P = 128
F = N // NCORES // P          # 2048
NB = 512                      # table bins per bucket table
EPS = 0.010                   # RDP fit tolerance per feature
NSLOT = 6                     # term tile ring
GCH = 32                      # gather chunks (ISA: <=1024 indices per call)
GF = 16 * F // GCH            # gather out free per chunk = 1024
GIC = F // GCH                # idxs columns consumed per gather chunk = 64
ICB = [0, 64, 512, 1024, 1536, 2048]   # idx compute chunk boundaries
HF = F // 2                   # output half
FEATS = ["lev", "wap", "cpn", "nav"]


def _bf16(a):
    import ml_dtypes
    return np.asarray(a, np.float32).astype(ml_dtypes.bfloat16)


def _hinge_np(x, knots, w, b):
    return np.maximum(x[:, None] - np.asarray(knots)[None, :], 0.0) \
        @ np.asarray(w) + b


def _rdp_fit(knots, w, b, lo, hi, eps):
    """Refit hinge as const + sum wts*relu(x - taus) with fewer knots,
    L_inf error <= eps on [lo, hi]."""
    knots = np.asarray(knots, np.float64)
    w = np.asarray(w, np.float64)
    pts = np.array([lo] + [t for t in knots if lo < t < hi] + [hi])
    vals = _hinge_np(pts, knots, w, float(b))
    keep = np.zeros(len(pts), bool)
    keep[0] = keep[-1] = True

    def recurse(i, j):
        if j <= i + 1:
            return
        t = (pts[i + 1:j] - pts[i]) / (pts[j] - pts[i])
        line = vals[i] + t * (vals[j] - vals[i])
        dev = np.abs(vals[i + 1:j] - line)
        k = int(np.argmax(dev))
        if dev[k] > eps:
            keep[i + 1 + k] = True
            recurse(i, i + 1 + k)
            recurse(i + 1 + k, j)

    recurse(0, len(pts) - 1)
    kp = np.where(keep)[0]
    xs, ys = pts[kp], vals[kp]
    slopes = np.diff(ys) / np.diff(xs)
    taus = [xs[0]]
    wts = [slopes[0]]
    for i in range(1, len(slopes)):
        taus.append(xs[i])
        wts.append(slopes[i] - slopes[i - 1])
    return np.array(taus), np.array(wts), float(ys[0])


def _build_program(m_per_feat, signs):
    """m_per_feat: tuple of knot counts per feature; signs: +/-1 per knot
    (concatenated over features in order)."""
    m_tot = sum(m_per_feat)
    ppw = 4 + 2 * m_tot
    nc = bass.Bass(detect_race_conditions=False)

    mv_in = nc.declare_dram_parameter("mv", [P, F], DT.float32, isOutput=False)
    bq_in = nc.declare_dram_parameter("bq", [P, F], DT.float32, isOutput=False)
    x_in = {f: nc.declare_dram_parameter(f, [P, F], DT.bfloat16, isOutput=False)
            for f in FEATS}
    tc_in = nc.declare_dram_parameter("tc", [P, 2 * NB], DT.bfloat16,
                                      isOutput=False)
    pp_in = nc.declare_dram_parameter("pp", [P, ppw], DT.float32, isOutput=False)
    ids_in = nc.declare_dram_parameter("ids", [P, 2 * P], DT.bfloat16,
                                       isOutput=False)
    y_out = nc.declare_dram_parameter("y", [P, F], DT.float32, isOutput=True)

    # knot j -> feature index
    kfeat = [fi for fi, m in enumerate(m_per_feat) for _ in range(m)]

    with ExitStack() as ctx:
        ec = ctx.enter_context
        mv = ec(nc.sbuf_tensor("mv_s", [P, F], DT.float32))
        bq = ec(nc.sbuf_tensor("bq_s", [P, F], DT.float32))
        xs = {f: ec(nc.sbuf_tensor(f"{f}_s", [P, F], DT.bfloat16)) for f in FEATS}
        tc = ec(nc.sbuf_tensor("tc_s", [P, 2 * NB], DT.bfloat16))
        pp = ec(nc.sbuf_tensor("pp_s", [P, ppw], DT.float32))
        ids = ec(nc.sbuf_tensor("ids_s", [P, 2 * P], DT.bfloat16))
        idx = ec(nc.sbuf_tensor("idx_s", [P, F], DT.uint16))
        G = [ec(nc.sbuf_tensor(f"g{i}_s", [P, GF], DT.bfloat16))
             for i in range(GCH)]
        V = ec(nc.sbuf_tensor("v_s", [P, F], DT.bfloat16))
        term = [ec(nc.sbuf_tensor(f"tm{i}_s", [P, F], DT.bfloat16))
                for i in range(NSLOT)]
        ysb = ec(nc.sbuf_tensor("y_s", [P, F], DT.float32))
        ps = ec(nc.psum_tensor("ps", [P, F], DT.float32))

        sem_a = ec(nc.semaphore())   # pp + mv/bq cols 0:64 loaded
        sem_b = ec(nc.semaphore())   # mv/bq cols 64:512 loaded
        sem_c = ec(nc.semaphore())   # mv/bq cols 512:2048 loaded
        sem_tc = ec(nc.semaphore())  # table loaded
        sem_x = [ec(nc.semaphore(name=f"sem_x{i}")) for i in range(len(FEATS))]
        sem_id = ec(nc.semaphore())  # identity tiles loaded
        sem_y = ec(nc.semaphore())   # y written back
        esem = [ec(nc.semaphore(name=f"esem{i}")) for i in range(2)]
        isem = ec(nc.semaphore())    # idx chunks ready (vector)
        gsem = ec(nc.semaphore())    # gather chunks done (gpsimd)
        tsem = ec(nc.semaphore())    # term tiles ready (vector)
        psem = ec(nc.semaphore())    # matmuls done (tensor), 4 per term
        fsem = ec(nc.semaphore())    # final y ready
        blk = ec(nc.Block())

        @blk.sync
        def _(s):
            s.dma_start(out=pp[:], in_=pp_in[:]).then_inc(sem_a, 16)
            for t_, t_in in [(mv, mv_in), (bq, bq_in)]:
                s.dma_start(out=t_[:, 0:64], in_=t_in[:, 0:64]).then_inc(sem_a, 16)
            s.dma_start(out=tc[:], in_=tc_in[:]).then_inc(sem_tc, 16)
            s.dma_start(out=xs["lev"][:], in_=x_in["lev"][:]).then_inc(sem_x[0], 16)
            for t_, t_in in [(mv, mv_in), (bq, bq_in)]:
                s.dma_start(out=t_[:, 64:512], in_=t_in[:, 64:512]).then_inc(sem_b, 16)
            s.dma_start(out=xs["wap"][:], in_=x_in["wap"][:]).then_inc(sem_x[1], 16)
            for t_, t_in in [(mv, mv_in), (bq, bq_in)]:
                s.dma_start(out=t_[:, 512:F], in_=t_in[:, 512:F]).then_inc(sem_c, 16)
            s.dma_start(out=xs["cpn"][:], in_=x_in["cpn"][:]).then_inc(sem_x[2], 16)
            s.dma_start(out=xs["nav"][:], in_=x_in["nav"][:]).then_inc(sem_x[3], 16)
            s.dma_start(out=ids[:], in_=ids_in[:]).then_inc(sem_id, 16)
            s.wait_ge(fsem, 1)
            s.dma_start(out=y_out[:, 0:HF], in_=ysb[:, 0:HF]).then_inc(sem_y, 16)
            s.wait_ge(fsem, 2)
            s.dma_start(out=y_out[:, HF:F], in_=ysb[:, HF:F]).then_inc(sem_y, 16)
            s.wait_ge(sem_y, 32)

        @blk.vector
        def _(v):
            # idx chunks: idx = u16(mv*invd + bq), bq = cofs + NB*slotbit
            for ci in range(len(ICB) - 1):
                v.wait_ge(sem_a, 48)
                if ci >= 1:
                    v.wait_ge(sem_b, 32)
                if ci >= 2:
                    v.wait_ge(sem_c, 32)
                sl = slice(ICB[ci], ICB[ci + 1])
                nc.vector.scalar_tensor_tensor(
                    out=idx[:, sl], in0=mv[:, sl], scalar=pp[:, 0:1],
                    op0=ALU.mult, in1=bq[:, sl], op1=ALU.add).then_inc(isem, 1)
            # hinge terms: max(|w|*x, |w|*t)
            for j in range(m_tot):
                v.wait_ge(sem_x[kfeat[j]], 16)
                if j >= NSLOT:
                    v.wait_ge(psem, 4 * (j - NSLOT + 1))
                nc.vector.tensor_scalar(
                    out=term[j % NSLOT][:], in0=xs[FEATS[kfeat[j]]][:],
                    scalar1=pp[:, 4 + 2 * j:5 + 2 * j],
                    scalar2=pp[:, 5 + 2 * j:6 + 2 * j],
                    op0=ALU.mult, op1=ALU.max).then_inc(tsem, 1)
            # final: y = psum + V, by column half (half h <- ext parity h)
            v.wait_ge(psem, 4 * m_tot)
            for h in range(2):
                v.wait_ge(esem[h], 16 * (GCH // 2))
                sl = slice(h * HF, (h + 1) * HF)
                nc.vector.tensor_tensor(out=ysb[:, sl], in0=ps[:, sl],
                                        in1=V[:, sl],
                                        op=ALU.add).then_inc(fsem, 1)

        # evens first so V's first column half completes early
        gorder = list(range(0, GCH, 2)) + list(range(1, GCH, 2))

        def _ic(c):
            # idx compute chunk covering gather chunk c (cols 64c..64c+64)
            lo = 64 * c
            for i in range(len(ICB) - 1):
                if lo < ICB[i + 1]:
                    return i
            raise AssertionError

        @blk.gpsimd
        def _(g):
            g.wait_ge(sem_tc, 16)
            for c in gorder:
                g.wait_ge(isem, _ic(c) + 1)
                nc.gpsimd.indirect_copy(
                    out=G[c][:], data=tc[:],
                    idxs=idx[:, c * GIC:(c + 1) * GIC],
                    i_know_ap_gather_is_preferred=True).then_inc(gsem, 1)

        @blk.scalar
        def _(sc):
            # de-replicate gather output; chunk c holds positions
            # [1024c, 1024(c+1)) of each group: q = c//2, col half c%2
            for k, c in enumerate(gorder):
                sc.wait_ge(gsem, k + 1)
                q, h = c // 2, c % 2
                sc.dma_start(
                    out=V[q:P:16, h * GF:(h + 1) * GF],
                    in_=G[c][q:P:16, :],
                ).then_inc(esem[h], 16)

        @blk.tensor
        def _(t):
            t.wait_ge(sem_id, 16)   # ids loaded
            for j in range(m_tot):
                t.wait_ge(tsem, j + 1)
                lt = ids[:, 0:P] if signs[j] > 0 else ids[:, P:2 * P]
                for b in range(4):
                    nc.tensor.matmul(
                        out=ps[:, 512 * b:512 * (b + 1)],
                        lhsT=lt,
                        rhs=term[j % NSLOT][:, 512 * b:512 * (b + 1)],
                        start=(j == 0), stop=(j == m_tot - 1),
                        skip_group_check=True,
                    ).then_inc(psem, 1)

    return nc


_CACHE = {}


def _get_program(m_per_feat, signs):
    key = (tuple(m_per_feat), tuple(signs))
    if key not in _CACHE:
        _CACHE[key] = _build_program(key[0], key[1])
    return _CACHE[key]


def _permute_gather_layout(nat):
    """nat: [P, F] per-core tile -> wrapped layout so that the elementwise
    idx tile is the gather's wrapped index stream AND extraction slices are
    contiguous. perm[16g + f%16, q*128 + f//16] = nat[16g + q, f]."""
    a = nat.reshape(8, 16, F // 16, 16)          # [g, q, f16, s]
    return np.ascontiguousarray(
        a.transpose(0, 3, 1, 2).reshape(P, F))   # [g, s, q, f16]


def kernel(**inputs):
    inp = {k: np.asarray(v) for k, v in inputs.items()}
    bidx_raw = inp["bucket_idx"].reshape(-1).astype(np.int64)
    # bucket-sorted sharding: each (core, partition) slot of F elements
    # spans at most 2 buckets
    order = np.argsort(bidx_raw, kind="stable")
    bidx = bidx_raw[order]
    mvoc = inp["mvoc"].astype(np.float64)[order]

    feat_arr = {"lev": inp["lev_idx"], "wap": inp["wap"],
                "cpn": inp["cpnspread"], "nav": inp["equity_nav"]}
    feat_par = {"lev": (inp["idx_knots"], inp["idx_w"], inp["idx_b"]),
                "wap": (inp["wap_knots"], inp["wap_w"], inp["wap_b"]),
                "cpn": (inp["cpn_knots"], inp["cpn_w"], inp["cpn_b"]),
                "nav": (inp["nav_knots"], inp["nav_w"], inp["nav_b"])}

    # --- fit part-B hinges on actual data ranges ---
    consts = float(inp["bias"])
    taus_all, wts_all = [], []
    m_per_feat = []
    for f in FEATS:
        x = feat_arr[f].astype(np.float64)
        kn, w, b = feat_par[f]
        taus, wts, c0 = _rdp_fit(kn, w, float(b), float(x.min()) - 1e-6,
                                 float(x.max()) + 1e-6, EPS)
        consts += c0
        m_per_feat.append(len(taus))
        taus_all.append(taus)
        wts_all.append(wts)
    taus_cat = np.concatenate(taus_all)
    wts_cat = np.concatenate(wts_all)
    signs = tuple(1 if w >= 0 else -1 for w in wts_cat)
    # max-trick constant correction: term contributes s*|w|*t extra
    consts -= float(np.sum(np.sign(wts_cat) * np.abs(wts_cat) * taus_cat))

    # --- part-A bucket tables over actual mvoc range ---
    lo_raw, hi_raw = float(mvoc.min()), float(mvoc.max())
    delta = (hi_raw - lo_raw) / (NB - 2) * (1 + 1e-6)
    lo = lo_raw - 0.5 * delta
    grid = lo + delta * np.arange(NB)
    base = _hinge_np(grid, inp["base_knots"].astype(np.float64),
                     inp["base_w"].astype(np.float64), float(inp["base_b"]))
    Tb = np.zeros((16, NB))
    for b in range(16):
        Tb[b] = base + _hinge_np(grid, inp["adj_knots"][b].astype(np.float64),
                                 inp["adj_w"][b].astype(np.float64),
                                 float(inp["adj_b"][b])) + consts
    Tb = _bf16(Tb)

    # per-slot (core, partition) bucket pair + slotbit stream
    slot_first = bidx[0::F]                      # [1024]
    slot_last = bidx[F - 1::F]                   # [1024]
    slotbit = (bidx != np.repeat(slot_first, F)).astype(np.float32)
    invd = np.float32(1.0 / delta)
    cofs = np.float32(-lo / delta)
    bq_full = (cofs + np.float32(NB) * slotbit).astype(np.float32)

    # --- params tile (same for all cores) ---
    m_tot = int(sum(m_per_feat))
    ppw = 4 + 2 * m_tot
    pp = np.zeros(ppw, np.float32)
    pp[0] = invd
    pp[4:4 + 2 * m_tot:2] = np.abs(wts_cat)
    pp[5:5 + 2 * m_tot:2] = np.abs(wts_cat) * taus_cat
    pp_tile = np.ascontiguousarray(np.broadcast_to(pp, (P, ppw)))

    ids_tile = np.ascontiguousarray(np.concatenate(
        [_bf16(np.eye(P, dtype=np.float32)),
         _bf16(-np.eye(P, dtype=np.float32))], axis=1))

    mv_f32 = mvoc.astype(np.float32)
    fx_bf = {f: _bf16(np.asarray(feat_arr[f], np.float32)[order]) for f in FEATS}

    NC_ROWS = P * F
    in_maps = []
    for c in range(NCORES):
        sl = slice(c * NC_ROWS, (c + 1) * NC_ROWS)
        # per-slot table pairs for this core
        sf = slot_first[c * P:(c + 1) * P]
        sla = slot_last[c * P:(c + 1) * P]
        tc_tile = np.empty((P, 2 * NB), Tb.dtype)
        tc_tile[:, 0:NB] = Tb[sf]
        tc_tile[:, NB:2 * NB] = Tb[sla]
        m = {
            "mv": _permute_gather_layout(mv_f32[sl].reshape(P, F)),
            "bq": _permute_gather_layout(bq_full[sl].reshape(P, F)),
            "tc": tc_tile,
            "pp": pp_tile,
            "ids": ids_tile,
        }
        for f in FEATS:
            m[f] = np.ascontiguousarray(fx_bf[f][sl].reshape(P, F))
        in_maps.append(m)

    nc = _get_program(m_per_feat, signs)
    res = run_bass_kernel_spmd(nc, in_maps, list(range(NCORES)))
    y_sorted = np.empty((N,), np.float32)
    for c in range(NCORES):
        y_sorted[c * NC_ROWS:(c + 1) * NC_ROWS] = res.results[c]["y"].reshape(-1)
    out = np.empty((N,), np.float32)
    out[order] = y_sorted
    return out
